# revision 24
# baseline (speedup 1.0000x reference)
"""Multi-head attention (RoPE, causal) Trainium2 Bass kernel.

Sharding: 8 cores = DP(2 batches) x TP(4 head-quads of 4 heads each).
Each core computes, for its batch b and head block hq (heads 4hq..4hq+3):
  q/k/v projections (bf16 matmuls), RoPE on q/k, causal attention in
  "scoresT" orientation (scores[sk, sq]), unnormalized ctx + sumexp via
  ones-matmul, normalization, and its partial slice of the output
  projection.  Host sums the 4 TP partials per batch and adds bo.

kernel(**inputs) takes the FULL unsharded inputs (numpy, keyed as in
setup_inputs) and returns the FULL [B, S, D] float32 output.
"""

import sys

if "/opt/trn_rl_repo" not in sys.path:
    sys.path.insert(0, "/opt/trn_rl_repo")

import numpy as np
import ml_dtypes

import concourse.bass as bass
import concourse.bacc as bacc
import concourse.mybir as mybir
import concourse.tile as tile
from concourse.bass_utils import run_bass_kernel_spmd

BF16 = mybir.dt.bfloat16
F32 = mybir.dt.float32
NPBF16 = ml_dtypes.bfloat16

B, S, D, H, DK = 2, 2048, 1024, 16, 64
NCORES = 8
TP = 4            # head-quads per batch
HPC = H // TP     # heads per core = 4
OC = HPC * DK     # output dims per core for q/k/v projections = 256
NPAIR = HPC // 2  # head pairs per core = 2
NB = S // 512     # sq blocks of width 512
NT = S // 128     # sk tiles of width 128
ND = D // 128     # contraction d-tiles

last_exec_time_ns = None
_cache = {}


def _rope_tables():
    """COS/SSIN tables [128, S]: rows j in 0:32 = cos/-sin of freq j,
    rows 32:64 = cos/+sin, repeated for the 2nd head of the pair."""
    a = np.arange(0, DK, 2, dtype=np.float32)
    inv_freq = (10000.0 ** (-2.0 * a / DK)).astype(np.float32)  # [32]
    pos = np.arange(S, dtype=np.float32)
    ang = pos[:, None] * inv_freq[None, :]          # [S, 32]
    cos = np.cos(ang).T.astype(np.float32)          # [32, S]
    sin = np.sin(ang).T.astype(np.float32)
    cos128 = np.concatenate([cos, cos, cos, cos], axis=0)     # [128, S]
    # signs baked per-row for the shifted-output t2 formulation:
    # t2[e-rows] reads ssin[o-rows] -> needs -sin; t2[o-rows] reads
    # ssin[e-rows] -> needs +sin.
    sin128 = np.concatenate([sin, -sin, sin, -sin], axis=0)   # [128, S]
    return cos128, sin128


def _analyze_mask(mask):
    """Classify [sk_tile 128] x [sq_block 512] blocks of the attention mask.

    Returns (blocks, tiles_w, tiles_n):
      blocks[b] = list of (t, l, kind, mid) for sk tiles not fully blocked:
        l    = count of leading sq columns that are fully blocked (those are
               skipped entirely: scores/exp/ctx all start at column l),
        kind = 0 fully allowed beyond l (no mask multiply),
               1 narrow: columns [l, l+128) partially masked, rest allowed;
                 mid indexes tiles_n ([128, 128] bf16 multiplier),
               2 wide fallback; mid indexes tiles_w ([128, 512], l is 0).
    """
    m = np.asarray(mask).reshape(S, S)  # [sq, sk], nonzero = allowed
    blocks = []
    tiles_w, keys_w = [], {}
    tiles_n, keys_n = [], {}
    for b in range(NB):
        cur = []
        for t in range(NT):
            tl = (m[512 * b:512 * b + 512, 128 * t:128 * t + 128] != 0).T
            # tl: [128 sk, 512 sq]
            if not tl.any():
                continue
            if tl.all():
                cur.append((t, 0, 0, None))
                continue
            colfull = tl.all(axis=0)   # fully allowed sq cols
            colany = tl.any(axis=0)
            l = 0
            while l < 512 and not colany[l]:
                l += 1
            nonfull = np.nonzero(~colfull)[0]
            r = int(nonfull[-1]) + 1 if len(nonfull) else l
            if tl[:, l:].all():
                cur.append((t, l, 0, None))
                continue
            if r - l <= 128:
                w = min(128, 512 - l)
                sub = np.ones((128, 128), NPBF16)
                sub[:, 0:w] = tl[:, l:l + w].astype(NPBF16)
                k = sub.tobytes()
                if k not in keys_n:
                    keys_n[k] = len(tiles_n)
                    tiles_n.append(sub)
                cur.append((t, l, 1, keys_n[k]))
            else:
                sub = tl.astype(NPBF16)
                k = sub.tobytes()
                if k not in keys_w:
                    keys_w[k] = len(tiles_w)
                    tiles_w.append(sub)
                cur.append((t, 0, 2, keys_w[k]))
        blocks.append(cur)
    return blocks, tiles_w, tiles_n


def _build_nc(blocks, n_masks_w, n_masks_n, qk_bias=False, v_bias=False,
              stage=5, loop_n=None, abl=()):
    nc = bacc.Bacc(None)

    xq = nc.declare_dram_parameter("xqT", [D, S], BF16, isOutput=False)
    xk = nc.declare_dram_parameter("xkT", [D, S], BF16, isOutput=False)
    xv = nc.declare_dram_parameter("xvT", [D, S], BF16, isOutput=False)
    wq = nc.declare_dram_parameter("wqT", [D, OC], BF16, isOutput=False)
    wk = nc.declare_dram_parameter("wkT", [D, OC], BF16, isOutput=False)
    wv = nc.declare_dram_parameter("wvT", [D, OC], BF16, isOutput=False)
    wo = nc.declare_dram_parameter("woT", [OC, D], BF16, isOutput=False)
    cosd = nc.declare_dram_parameter("cos", [128, S], BF16, isOutput=False)
    ssind = nc.declare_dram_parameter("ssin", [128, S], BF16, isOutput=False)
    bqd = nc.declare_dram_parameter("bq", [128, NPAIR], F32, isOutput=False)
    bkd = nc.declare_dram_parameter("bk", [128, NPAIR], F32, isOutput=False)
    bvd = nc.declare_dram_parameter("bv", [128, OC], F32, isOutput=False)
    nm_w = max(n_masks_w, 1)
    nm_n = max(n_masks_n, 1)
    maskd = nc.declare_dram_parameter("masks", [nm_w, 128, 512], BF16,
                                      isOutput=False)
    maskn = nc.declare_dram_parameter("masksn", [nm_n, 128, 128], BF16,
                                      isOutput=False)
    outp = nc.declare_dram_parameter("out", [S, D], F32, isOutput=True)

    with tile.TileContext(nc) as tc:
        from contextlib import ExitStack, nullcontext
        with ExitStack() as ctx:
            ep = ctx.enter_context
            const = ep(tc.tile_pool(name="const", bufs=1))
            xt_p = ep(tc.tile_pool(name="xt", bufs=12))
            xv_p = ep(tc.tile_pool(name="xv", bufs=16))
            w_p = ep(tc.tile_pool(name="w", bufs=24))
            rope_p = ep(tc.tile_pool(name="rope", bufs=6))
            hat_p = ep(tc.tile_pool(name="hat", bufs=4))
            vsb_p = ep(tc.tile_pool(name="vsb", bufs=17))
            e_p = ep(tc.tile_pool(name="e", bufs=8))
            ctx_p = ep(tc.tile_pool(name="ctxsb", bufs=6))
            rec_p = ep(tc.tile_pool(name="rec", bufs=2))
            out_p = ep(tc.tile_pool(name="outsb", bufs=6))
            sc_ps = ep(tc.tile_pool(name="sc", bufs=2, space="PSUM"))
            acc_ps = ep(tc.tile_pool(name="acc", bufs=4, space="PSUM"))
            if loop_n is not None:
                ep(tc.For_i(0, loop_n, 1))

            # ---- q/k input + weight DMAs first (PE can start ASAP) ----
            # x loaded in half-S chunks, ordered q-h0, k-h0, q-h1, k-h1 so
            # attention blocks 0-1 (which need only sk/sq < 1024) can start
            # while the second halves are still loading.
            xts, wts = {}, {}
            for h in (0, 1):
                for name, xd in (("q", xq), ("k", xk)):
                    for dt in range(ND):
                        x_t = xt_p.tile([128, S // 2], BF16, tag="xt")
                        nc.sync.dma_start(
                            out=x_t,
                            in_=xd[128 * dt:128 * dt + 128,
                                   1024 * h:1024 * h + 1024])
                        xts[(name, dt, h)] = x_t
            for name, wd in (("q", wq), ("k", wk)):
                wt = []
                for dt in range(ND):
                    w_t = w_p.tile([128, OC], BF16, tag="w")
                    nc.gpsimd.dma_start(out=w_t, in_=wd[128 * dt:128 * dt + 128, :])
                    wt.append(w_t)
                wts[name] = wt
            vw = []
            for dt in range(ND):
                w_t = w_p.tile([128, OC], BF16, tag="w")
                nc.gpsimd.dma_start(out=w_t, in_=wv[128 * dt:128 * dt + 128, :])
                vw.append(w_t)

            # ---- constants ----
            cos_sb = const.tile([128, S], BF16)
            ssin_sb = const.tile([128, S], BF16)
            nc.gpsimd.dma_start(out=cos_sb, in_=cosd[:, :])
            nc.gpsimd.dma_start(out=ssin_sb, in_=ssind[:, :])
            wo_sb = []
            for p in range(NPAIR):
                w_t = const.tile([128, D], BF16, tag=f"wo{p}")
                nc.gpsimd.dma_start(out=w_t, in_=wo[128 * p:128 * p + 128, :])
                wo_sb.append(w_t)
            bq_sb = const.tile([128, NPAIR], F32)
            bk_sb = const.tile([128, NPAIR], F32)
            if qk_bias:
                nc.gpsimd.dma_start(out=bq_sb, in_=bqd[:, :])
                nc.gpsimd.dma_start(out=bk_sb, in_=bkd[:, :])
            bv_sb = const.tile([128, OC], F32)
            if v_bias:
                nc.gpsimd.dma_start(out=bv_sb, in_=bvd[:, :])
            mask_sb = []
            for i in range(nm_w):
                m_t = const.tile([128, 512], BF16, tag=f"mask{i}")
                nc.gpsimd.dma_start(out=m_t, in_=maskd[i])
                mask_sb.append(m_t)
            maskn_sb = []
            for i in range(nm_n):
                m_t = const.tile([128, 128], BF16, tag=f"maskn{i}")
                nc.gpsimd.dma_start(out=m_t, in_=maskn[i])
                maskn_sb.append(m_t)

            # ---- q/k projections -> [o_p, s_f] + RoPE ----
            hats = {}
            for name, xd, wd, bias_sb in (("q", xq, wq, bq_sb),
                                          ("k", xk, wk, bk_sb)):
                xt = xts[name]
                wt = wts[name]
                for p in range(NPAIR):
                    raw = rope_p.tile([128, S], BF16, tag="raw")
                    for sb in range(4):
                        ps = acc_ps.tile([128, 512], F32, tag="acc")
                        for dt in range(ND):
                            nc.tensor.matmul(
                                ps,
                                lhsT=wt[dt][:, 128 * p:128 * p + 128],
                                rhs=xt[dt][:, 512 * sb:512 * sb + 512],
                                start=(dt == 0), stop=(dt == ND - 1))
                        # evict + per-partition bias -> bf16 (on ScalarE:
                        # it is idle during the projection phase and this
                        # keeps DVE free for RoPE)
                        if qk_bias:
                            tmp = rope_p.tile([128, 512], F32, tag="btmp")
                            nc.scalar.copy(tmp, ps)
                            nc.vector.tensor_scalar_add(
                                raw[:, 512 * sb:512 * sb + 512], tmp,
                                bias_sb[:, p:p + 1])
                        else:
                            nc.scalar.copy(
                                raw[:, 512 * sb:512 * sb + 512], ps)
                    # RoPE: hat[e] = raw[e]*cos - raw[o]*sin
                    #       hat[o] = raw[o]*cos + raw[e]*sin
                    # t2 written with partition-SHIFTED outputs (inputs stay
                    # aligned; sign baked into the ssin table rows), then one
                    # full-width add.  Shifted-output DVE ops are
                    # probe-verified legal (walrus + CoreSim).
                    t1 = hat_p.tile([128, S], BF16, tag="hat")
                    nc.vector.tensor_mul(t1, raw, cos_sb)
                    t2 = rope_p.tile([128, S], BF16, tag="t2")
                    nc.vector.tensor_mul(t2[0:32, :], raw[32:64, :],
                                         ssin_sb[32:64, :])
                    nc.vector.tensor_mul(t2[32:64, :], raw[0:32, :],
                                         ssin_sb[0:32, :])
                    nc.vector.tensor_mul(t2[64:96, :], raw[96:128, :],
                                         ssin_sb[96:128, :])
                    nc.vector.tensor_mul(t2[96:128, :], raw[64:96, :],
                                         ssin_sb[64:96, :])
                    nc.vector.tensor_add(t1, t1, t2)
                    hats[(name, p)] = t1

            # ---- v projection (by 512-col s-chunk, interleaved with the
            # attention blocks so exp work starts early) ----
            # vsb layout per tile: [A0 | ones | B0 | A1 | ones | B1] (384
            # cols).  The attention ctx matmul then uses contiguous 128-col
            # lhsT slices [head|ones] / [ones|head], so one e-stream yields
            # both the ctx rows and the softmax-denominator rows — no
            # separate ones-matmul.
            if stage < 2:
                nc.finalize2 = None  # placeholder

            # src blocks {0,64,128,192} -> dst blocks {0,128,192,320}: one
            # 4D-AP copy ([pair 128->192][blk 64->128][64]).
            def vmap4(ap384):
                return (ap384.rearrange("p (a c) -> p a c", a=2)
                        .rearrange("p a (b c) -> p a b c", c=64)[:, :, 0:3:2, :])

            def psrc4(ap256):
                return ap256.rearrange("p (a b c) -> p a b c", a=2, c=64)

            vsb = [None] * NT

            def v_chunk(ci):
                xvt = []
                for dt in range(ND):
                    x_t = xv_p.tile([128, 512], BF16, tag="xv")
                    nc.sync.dma_start(
                        out=x_t,
                        in_=xv[128 * dt:128 * dt + 128,
                               512 * ci:512 * ci + 512])
                    xvt.append(x_t)
                for sti in range(4):
                    st = 4 * ci + sti
                    ps = acc_ps.tile([128, 512], F32, tag="acc")
                    for dt in range(ND):
                        nc.tensor.matmul(
                            ps[:, 0:OC],
                            lhsT=xvt[dt][:, 128 * sti:128 * sti + 128],
                            rhs=vw[dt][:, :],
                            start=(dt == 0), stop=(dt == ND - 1))
                    v_t = vsb_p.tile([128, 384], BF16, tag="vsb")
                    if v_bias:
                        nc.vector.tensor_add(vmap4(v_t[:, 0:384]),
                                             psrc4(ps[:, 0:OC]),
                                             psrc4(bv_sb[:, 0:OC]))
                    else:
                        nc.vector.tensor_copy(vmap4(v_t[:, 0:384]),
                                              psrc4(ps[:, 0:OC]))
                    nc.gpsimd.memset(v_t[:, 64:128], 1.0)
                    nc.gpsimd.memset(v_t[:, 256:320], 1.0)
                    vsb[st] = v_t

            # ---- attention + output projection ----
            for b in (range(NB) if stage >= 3 else []):
                v_chunk(b)
                act = blocks[b]
                ctxsb = []
                for p in range(NPAIR):
                    qh = hats[("q", p)]
                    kh = hats[("k", p)]
                    # ps_a: rows 0:64 ctx of head A, rows 64:128 denom A.
                    # ps_b: rows 0:64 denom B, rows 64:128 ctx of head B.
                    ps_a = acc_ps.tile([128, 512], F32, tag="acc")
                    ps_b = acc_ps.tile([128, 512], F32, tag="acc")
                    n = len(act)
                    gi = 0
                    for g0 in range(0, n, 2):
                        grp = act[g0:g0 + 2]
                        e0 = e_p.tile([128, 1024], BF16, tag="e")
                        e1 = e_p.tile([128, 1024], BF16, tag="e")
                        ps0 = sc_ps.tile([128, 1024], F32, tag="sc")
                        ps1 = sc_ps.tile([128, 1024], F32, tag="sc")
                        for c, (t, l, kind, mid) in enumerate(grp):
                            sl = slice(512 * c + l, 512 * c + 512)
                            sq = slice(512 * b + l, 512 * b + 512)
                            nc.tensor.matmul(
                                ps0[:, sl],
                                lhsT=kh[0:64, 128 * t:128 * t + 128],
                                rhs=qh[0:64, sq],
                                start=True, stop=True, tile_position=(0, 0))
                            nc.tensor.matmul(
                                ps1[:, sl],
                                lhsT=kh[64:128, 128 * t:128 * t + 128],
                                rhs=qh[64:128, sq],
                                start=True, stop=True, tile_position=(64, 0))
                        if all(l == 0 for (t, l, kind, mid) in grp):
                            wdt = 512 * len(grp)
                            nc.scalar.activation(
                                e0[:, 0:wdt], ps0[:, 0:wdt],
                                mybir.ActivationFunctionType.Exp)
                            nc.scalar.activation(
                                e1[:, 0:wdt], ps1[:, 0:wdt],
                                mybir.ActivationFunctionType.Exp)
                        else:
                            for c, (t, l, kind, mid) in enumerate(grp):
                                sl = slice(512 * c + l, 512 * c + 512)
                                nc.scalar.activation(
                                    e0[:, sl], ps0[:, sl],
                                    mybir.ActivationFunctionType.Exp)
                                nc.scalar.activation(
                                    e1[:, sl], ps1[:, sl],
                                    mybir.ActivationFunctionType.Exp)
                        for c, (t, l, kind, mid) in enumerate(grp):
                            if kind == 0 or "nodiag" in abl:
                                continue
                            if kind == 1:
                                w = min(128, 512 - l)
                                sl = slice(512 * c + l, 512 * c + l + w)
                                msk = maskn_sb[mid][:, 0:w]
                            else:
                                sl = slice(512 * c, 512 * c + 512)
                                msk = mask_sb[mid]
                            nc.vector.tensor_mul(e0[:, sl], e0[:, sl], msk)
                            nc.vector.tensor_mul(e1[:, sl], e1[:, sl], msk)
                        for c, (t, l, kind, mid) in enumerate(grp):
                            if stage < 4:
                                gi += 1
                                continue
                            sl = slice(512 * c + l, 512 * c + 512)
                            out_sl = slice(l, 512)
                            first = (gi == 0)
                            last = (gi == n - 1)
                            nc.tensor.matmul(
                                ps_a[:, out_sl],
                                lhsT=vsb[t][:, 192 * p:192 * p + 128],
                                rhs=e0[:, sl], start=first, stop=last)
                            nc.tensor.matmul(
                                ps_b[:, out_sl],
                                lhsT=vsb[t][:, 192 * p + 64:192 * p + 192],
                                rhs=e1[:, sl], start=first, stop=last)
                            gi += 1
                    if stage < 4:
                        continue
                    csb = ctx_p.tile([128, 512], BF16, tag="ctxsb")
                    rec = rec_p.tile([128, 512], F32, tag="rec")
                    nc.vector.reciprocal(rec[0:64, :], ps_a[64:128, :])
                    nc.vector.reciprocal(rec[64:128, :], ps_b[0:64, :])
                    nc.vector.tensor_mul(csb[0:64, :], ps_a[0:64, :],
                                         rec[0:64, :])
                    nc.vector.tensor_mul(csb[64:128, :], ps_b[64:128, :],
                                         rec[64:128, :])
                    ctxsb.append(csb)

                for j in (range(4) if stage >= 5 else []):
                    for oh in range(2):
                        ps = acc_ps.tile([128, 512], F32, tag="acc")
                        for p in range(NPAIR):
                            nc.tensor.matmul(
                                ps,
                                lhsT=ctxsb[p][:, 128 * j:128 * j + 128],
                                rhs=wo_sb[p][:, 512 * oh:512 * oh + 512],
                                start=(p == 0), stop=(p == NPAIR - 1))
                        o_t = out_p.tile([128, 512], F32, tag="outsb")
                        nc.vector.tensor_copy(o_t, ps)
                        nc.gpsimd.dma_start(
                            out=outp[512 * b + 128 * j:512 * b + 128 * j + 128,
                                     512 * oh:512 * oh + 512],
                            in_=o_t)
    nc.finalize()
    return nc


def _prep_core_inputs(inputs, blocks, tiles_w, tiles_n):
    """Build the 8 per-core input maps (host-side sharding)."""
    q = np.asarray(inputs["q"], np.float32)
    k = np.asarray(inputs["k"], np.float32)
    v = np.asarray(inputs["v"], np.float32)
    Wq = np.asarray(inputs["Wq"], np.float32)
    Wk = np.asarray(inputs["Wk"], np.float32)
    Wv = np.asarray(inputs["Wv"], np.float32)
    Wo = np.asarray(inputs["Wo"], np.float32)
    bq = np.asarray(inputs["bq"], np.float32)
    bk = np.asarray(inputs["bk"], np.float32)
    bv = np.asarray(inputs["bv"], np.float32)

    cos128, ssin128 = _rope_tables()
    cos_b = cos128.astype(NPBF16)
    ssin_b = ssin128.astype(NPBF16)
    nm_w = max(len(tiles_w), 1)
    masks_t = np.zeros((nm_w, 128, 512), NPBF16)
    for i, t in enumerate(tiles_w):
        masks_t[i] = t
    nm_n = max(len(tiles_n), 1)
    masksn_t = np.zeros((nm_n, 128, 128), NPBF16)
    for i, t in enumerate(tiles_n):
        masksn_t[i] = t

    # de-interleave permutation within each head: evens then odds
    perm64 = np.concatenate([np.arange(0, DK, 2), np.arange(1, DK, 2)])

    xT = {}
    for bb in range(B):
        xT[("q", bb)] = np.ascontiguousarray(q[bb].T).astype(NPBF16)
        xT[("k", bb)] = np.ascontiguousarray(k[bb].T).astype(NPBF16)
        xT[("v", bb)] = np.ascontiguousarray(v[bb].T).astype(NPBF16)

    scale = np.float32(1.0 / np.sqrt(DK))
    in_maps = []
    for c in range(NCORES):
        bb, hq = divmod(c, TP)
        rows = []
        for h in range(HPC):
            base = OC * hq + DK * h
            rows.extend((base + perm64).tolist())
        rows = np.array(rows)
        cols = np.arange(OC * hq, OC * hq + OC)

        wqT = np.ascontiguousarray(Wq[rows, :].T).astype(NPBF16)
        wkT = np.ascontiguousarray((Wk[rows, :] * scale).T).astype(NPBF16)
        wvT = np.ascontiguousarray(Wv[cols, :].T).astype(NPBF16)
        woT = np.ascontiguousarray(Wo[:, cols].T).astype(NPBF16)
        bq_t = np.ascontiguousarray(bq[rows].reshape(NPAIR, 128).T).astype(np.float32)
        bk_t = np.ascontiguousarray((bk[rows] * scale).reshape(NPAIR, 128).T).astype(np.float32)
        bv_t = np.broadcast_to(bv[cols], (128, OC)).astype(np.float32)

        in_maps.append({
            "xqT": xT[("q", bb)], "xkT": xT[("k", bb)], "xvT": xT[("v", bb)],
            "wqT": wqT, "wkT": wkT, "wvT": wvT, "woT": woT,
            "cos": cos_b, "ssin": ssin_b,
            "bq": bq_t, "bk": bk_t, "bv": bv_t,
            "masks": masks_t, "masksn": masksn_t,
        })
    return in_maps


def prepare(inputs, loop_n=None):
    """Build (nc, in_maps) for the given full inputs (test harness hook)."""
    mask = np.asarray(inputs["mask"])
    blocks, tiles_w, tiles_n = _analyze_mask(mask)
    qk_bias = bool(np.any(np.asarray(inputs["bq"])) or np.any(np.asarray(inputs["bk"])))
    v_bias = bool(np.any(np.asarray(inputs["bv"])))
    nc = _build_nc(blocks, len(tiles_w), len(tiles_n), qk_bias, v_bias,
                   loop_n=loop_n)
    in_maps = _prep_core_inputs(inputs, blocks, tiles_w, tiles_n)
    return nc, in_maps


def kernel(**inputs):
    global last_exec_time_ns
    import os

    mask = np.asarray(inputs["mask"])
    blocks, tiles_w, tiles_n = _analyze_mask(mask)
    qk_bias = bool(np.any(np.asarray(inputs["bq"])) or np.any(np.asarray(inputs["bk"])))
    v_bias = bool(np.any(np.asarray(inputs["bv"])))
    key = (tuple(tuple(bl) for bl in blocks), len(tiles_w), len(tiles_n),
           qk_bias, v_bias)
    if key not in _cache:
        _cache[key] = _build_nc(blocks, len(tiles_w), len(tiles_n),
                                qk_bias, v_bias)
    nc = _cache[key]

    in_maps = _prep_core_inputs(inputs, blocks, tiles_w, tiles_n)
    trace = bool(os.environ.get("KERNEL_TRACE"))
    import time
    last_err = None
    for attempt in range(3):
        try:
            res = run_bass_kernel_spmd(nc, in_maps, list(range(NCORES)),
                                       trace=trace)
            break
        except Exception as e:  # transient NRT device-unrecoverable wedges
            last_err = e
            time.sleep(10.0)
    else:
        raise last_err
    last_exec_time_ns = res.exec_time_ns

    bo = np.asarray(inputs["bo"], np.float32)
    out = np.zeros((B, S, D), np.float32)
    for c in range(NCORES):
        bb = c // TP
        out[bb] += res.results[c]["out"]
    out += bo[None, None, :]
    return out

